# revision 4
# baseline (speedup 1.0000x reference)
"""Trainium2 Bass kernel for BaseCausalWanSelfAttention (local+sink sparse attention
with interleaved rotary), SPMD across 8 NeuronCores.

Sharding: the 24 (batch, head) pairs are split 3-per-core across 8 cores; each
core runs full local+sink attention for its pairs independently (no collectives).
"""
import sys

sys.path.insert(0, "/opt/trn_rl_repo")

import numpy as np

import concourse.bacc as bacc
import concourse.tile as tile
import concourse.mybir as mybir

dt = mybir.dt

# Problem config (hardcoded per contest contract)
B, S, H, D = 2, 3072, 12, 128
LOCAL_WINDOW = 1560
SINK = 128
N_CORES = 8
PER_CORE = (B * H) // N_CORES  # 3
QB = 512  # query block (columns of transposed scores)
NQC = QB // 128  # 128-query chunks per block
SCALE = 1.0 / float(np.sqrt(D))


def _window_partial_deltas(w):
    """k-tile offsets (qi - kj) where the local-window edge cuts through the
    128x128 tile; maps delta -> threshold T with allowed iff (c - p) < T."""
    out = {}
    for d in range((w - 127 + 127) // 128, (w + 127) // 128 + 1):
        t = w - 128 * d
        if -127 <= t <= 127:
            out[d] = t
    return out


def chunk_kinds(qb, kj, w=LOCAL_WINDOW, nqc=NQC):
    """Per 128-query chunk classification of k-tile kj for query block qb.
    Returns list of (t, kind) with kind in {"full", "diag", ("win", delta)} for
    valid chunks only. SINK==128 assumed (k-tile 0 fully attendable)."""
    partial = _window_partial_deltas(w)
    max_delta = max(partial) if partial else (w - 1) // 128
    kinds = []
    for t in range(nqc):
        qi = nqc * qb + t
        if kj == 0:
            kinds.append((t, "diag" if qi == 0 else "full"))
            continue
        delta = qi - kj
        if delta < 0 or delta > max_delta:
            continue
        if delta == 0:
            kinds.append((t, "diag"))
        elif delta in partial:
            kinds.append((t, ("win", delta)))
        else:
            kinds.append((t, "full"))
    return kinds


def kj_list(qb, s=S, w=LOCAL_WINDOW, nqc=NQC):
    partial = _window_partial_deltas(w)
    max_delta = max(partial) if partial else (w - 1) // 128
    n_ktiles = s // 128
    hi = min(nqc * qb + nqc - 1, n_ktiles - 1)
    lo = max(1, nqc * qb - max_delta)
    return [0] + [kj for kj in range(lo, hi + 1)]


def build_nc(s=S, per_core=PER_CORE, w=LOCAL_WINDOW):
    """Build the SPMD single-core program (identical on all cores)."""
    nqb = s // QB
    partial = _window_partial_deltas(w)

    nc = bacc.Bacc("TRN2", target_bir_lowering=False, debug=False)

    qT = nc.declare_dram_parameter("qT", [per_core, 128, s], dt.float32r, isOutput=False)
    kT = nc.declare_dram_parameter("kT", [per_core, 128, s], dt.float32r, isOutput=False)
    v = nc.declare_dram_parameter("v", [per_core, s, 128], dt.float32r, isOutput=False)
    cexpT = nc.declare_dram_parameter("cexpT", [128, s], dt.float32r, isOutput=False)
    ssigT = nc.declare_dram_parameter("ssigT", [128, s], dt.float32r, isOutput=False)
    pswap = nc.declare_dram_parameter("pswap", [128, 128], dt.float32r, isOutput=False)
    ident = nc.declare_dram_parameter("ident", [128, 128], dt.float32, isOutput=False)
    ones1 = nc.declare_dram_parameter("ones1", [128, 1], dt.float32r, isOutput=False)
    maskD = nc.declare_dram_parameter("maskD", [128, 128], dt.float32r, isOutput=False)
    wmask_names = {}
    for delta in sorted(partial):
        nm = f"maskW{delta}"
        wmask_names[delta] = nc.declare_dram_parameter(
            nm, [128, 128], dt.float32r, isOutput=False
        )
    out = nc.declare_dram_parameter("out", [per_core, s, 128], dt.float32, isOutput=True)

    with tile.TileContext(nc) as tc:
        with (
            tc.tile_pool(name="const", bufs=1) as cpool,
            tc.tile_pool(name="big", bufs=2) as bigpool,
            tc.tile_pool(name="probs", bufs=6) as ppool,
            tc.tile_pool(name="tail", bufs=2) as tpool,
            tc.tile_pool(name="ps_sc", bufs=3, space="PSUM") as ps_sc,
            tc.tile_pool(name="ps_out", bufs=2, space="PSUM") as ps_out,
            tc.tile_pool(name="ps_den", bufs=1, space="PSUM") as ps_den,
            tc.tile_pool(name="ps_tr", bufs=2, space="PSUM") as ps_tr,
        ):
            # constants
            cexp_sb = cpool.tile([128, s], dt.float32r, tag="cexp")
            ssig_sb = cpool.tile([128, s], dt.float32r, tag="ssig")
            nc.sync.dma_start(out=cexp_sb[:], in_=cexpT[:])
            nc.sync.dma_start(out=ssig_sb[:], in_=ssigT[:])
            pswap_sb = cpool.tile([128, 128], dt.float32r, tag="pswap")
            ident_sb = cpool.tile([128, 128], dt.float32, tag="ident")
            ones1_sb = cpool.tile([128, 1], dt.float32r, tag="ones1")
            nc.sync.dma_start(out=pswap_sb[:], in_=pswap[:])
            nc.sync.dma_start(out=ident_sb[:], in_=ident[:])
            nc.sync.dma_start(out=ones1_sb[:], in_=ones1[:])
            maskD_sb = cpool.tile([128, 128], dt.float32r, tag="maskD")
            nc.sync.dma_start(out=maskD_sb[:], in_=maskD[:])
            wmask_sb = {}
            for delta, ap in wmask_names.items():
                m = cpool.tile([128, 128], dt.float32r, tag=f"maskW{delta}")
                nc.sync.dma_start(out=m[:], in_=ap[:])
                wmask_sb[delta] = m

            def prep(u):
                """Load + rotary for unit u; returns (rq, rk, v_sb)."""
                qraw = bigpool.tile([128, s], dt.float32r, tag="qraw")
                kraw = bigpool.tile([128, s], dt.float32r, tag="kraw")
                v_sb = bigpool.tile([128, s], dt.float32r, tag="v")
                nc.sync.dma_start(out=qraw[:], in_=qT[u])
                nc.sync.dma_start(out=kraw[:], in_=kT[u])
                nc.sync.dma_start(
                    out=v_sb[:].rearrange("p (n d) -> p n d", d=128),
                    in_=v[u].rearrange("(n p) d -> p n d", p=128),
                )
                rots = {}
                for name, raw in (("q", qraw), ("k", kraw)):
                    r = bigpool.tile([128, s], dt.float32r, tag=f"r{name}")
                    swaps = []
                    for c in range(s // 512):
                        sl = slice(c * 512, (c + 1) * 512)
                        sw = ps_sc.tile([128, 512], dt.float32, tag="sc")
                        nc.tensor.matmul(
                            sw[:], pswap_sb[:], raw[:, sl], start=True, stop=True
                        )
                        swaps.append((sl, sw))
                    # r = raw * cexp  (raw fully consumed after this)
                    nc.vector.tensor_mul(r[:], raw[:], cexp_sb[:])
                    # raw <- swap(raw) * ssig   (reuse raw as scratch)
                    for sl, sw in swaps:
                        nc.vector.tensor_mul(
                            raw[:, sl], sw[:].bitcast(dt.float32r), ssig_sb[:, sl]
                        )
                    # r += scratch  (gpsimd; sbuf-only operands)
                    nc.gpsimd.tensor_add(r[:], r[:], raw[:])
                    rots[name] = r
                return rots["q"], rots["k"], v_sb

            def attention(u, rq, rk, v_sb):
                for qb in range(nqb):
                    outT_ps = ps_out.tile([128, QB], dt.float32, tag="outT")
                    den_ps = ps_den.tile([1, QB], dt.float32, tag="den")
                    kjs = kj_list(qb, s=s, w=w)
                    for kj in kjs:
                        kinds = chunk_kinds(qb, kj, w=w)
                        assert kinds, (qb, kj)
                        t0 = kinds[0][0]
                        t1 = kinds[-1][0] + 1
                        csl = slice(qb * QB + t0 * 128, qb * QB + t1 * 128)
                        psl = slice(t0 * 128, t1 * 128)
                        ksl = slice(kj * 128, (kj + 1) * 128)
                        first = kj == 0
                        last = kj == kjs[-1]

                        sc = ps_sc.tile([128, QB], dt.float32, tag="sc")
                        nc.tensor.matmul(
                            sc[:, psl], rk[:, ksl], rq[:, csl], start=True, stop=True
                        )
                        probs = ppool.tile([128, QB], dt.float32r, tag="probs")
                        nc.scalar.activation(
                            probs[:, psl],
                            sc[:, psl],
                            mybir.ActivationFunctionType.Exp,
                            scale=SCALE,
                        )
                        for t, kind in kinds:
                            if kind == "full":
                                continue
                            m = maskD_sb if kind == "diag" else wmask_sb[kind[1]]
                            tsl = slice(t * 128, (t + 1) * 128)
                            nc.vector.tensor_mul(probs[:, tsl], probs[:, tsl], m[:])
                        nc.tensor.matmul(
                            outT_ps[:, psl],
                            v_sb[:, ksl],
                            probs[:, psl],
                            start=first,
                            stop=last,
                        )
                        nc.tensor.matmul(
                            den_ps[:, psl],
                            ones1_sb[:],
                            probs[:, psl],
                            start=first,
                            stop=last,
                        )

                    # ---- tail: normalize, transpose, store ----
                    rden = tpool.tile([1, QB], dt.float32, tag="rden")
                    nc.vector.reciprocal_approx_fast(rden[:], den_ps[:])
                    rdenb = tpool.tile([128, QB], dt.float32, tag="rdenb")
                    nc.gpsimd.partition_broadcast(rdenb[:], rden[:])
                    outN = tpool.tile([128, QB], dt.float32, tag="outN")
                    nc.vector.tensor_mul(outN[:], outT_ps[:], rdenb[:])
                    tr = ps_tr.tile([128, QB], dt.float32, tag="tr")
                    for c in range(NQC):
                        tsl = slice(c * 128, (c + 1) * 128)
                        nc.tensor.transpose(tr[:, tsl], outN[:, tsl], ident_sb[:])
                    out_sb = tpool.tile([128, QB], dt.float32, tag="out_sb")
                    nc.scalar.copy(out_sb[:], tr[:])
                    nc.sync.dma_start(
                        out=out[u].rearrange("(n p) d -> p n d", p=128)[
                            :, qb * NQC : (qb + 1) * NQC, :
                        ],
                        in_=out_sb[:].rearrange("p (n d) -> p n d", d=128),
                    )

            prepped = prep(0)
            for u in range(per_core):
                nxt = prep(u + 1) if u + 1 < per_core else None
                attention(u, *prepped)
                prepped = nxt

    nc.compile()
    return nc


def host_prep(q, k, v, cos, sin, s=S, w=LOCAL_WINDOW):
    """Build per-core input maps from full inputs."""
    b, _, h, d = q.shape
    partial = _window_partial_deltas(w)

    cexp = np.empty((128, s), dtype=np.float32)
    ssig = np.empty((128, s), dtype=np.float32)
    cexp[0::2, :] = cos.T
    cexp[1::2, :] = cos.T
    ssig[0::2, :] = -sin.T
    ssig[1::2, :] = sin.T

    pswap = np.zeros((128, 128), dtype=np.float32)
    idx = np.arange(128)
    pswap[idx, idx ^ 1] = 1.0
    ident = np.eye(128, dtype=np.float32)
    ones1 = np.ones((128, 1), dtype=np.float32)

    p = np.arange(128)[:, None]
    c = np.arange(128)[None, :]
    maskD = (c >= p).astype(np.float32)
    wmasks = {
        delta: ((c - p) < t).astype(np.float32) for delta, t in partial.items()
    }

    units = [(bi, hi) for bi in range(b) for hi in range(h)]
    per = len(units) // N_CORES
    in_maps = []
    for core in range(N_CORES):
        us = units[core * per : (core + 1) * per]
        qTc = np.ascontiguousarray(
            np.stack([q[bi, :, hi, :].T for bi, hi in us])
        )
        kTc = np.ascontiguousarray(
            np.stack([k[bi, :, hi, :].T for bi, hi in us])
        )
        vc = np.ascontiguousarray(np.stack([v[bi, :, hi, :] for bi, hi in us]))
        m = {
            "qT": qTc,
            "kT": kTc,
            "v": vc,
            "cexpT": cexp,
            "ssigT": ssig,
            "pswap": pswap,
            "ident": ident,
            "ones1": ones1,
            "maskD": maskD,
        }
        for delta, msk in wmasks.items():
            m[f"maskW{delta}"] = msk
        in_maps.append(m)
    return in_maps, units


_NC_CACHE = {}


def kernel(q, k, v, cos, sin):
    from concourse.bass_utils import run_bass_kernel_spmd

    q = np.asarray(q, dtype=np.float32)
    k = np.asarray(k, dtype=np.float32)
    v = np.asarray(v, dtype=np.float32)
    cos = np.asarray(cos, dtype=np.float32)
    sin = np.asarray(sin, dtype=np.float32)

    if "nc" not in _NC_CACHE:
        _NC_CACHE["nc"] = build_nc()
    nc = _NC_CACHE["nc"]

    in_maps, units = host_prep(q, k, v, cos, sin)
    res = run_bass_kernel_spmd(nc, in_maps, core_ids=list(range(N_CORES)))

    b, s, h, d = q.shape
    full = np.empty((b, s, h, d), dtype=np.float32)
    per = len(units) // N_CORES
    for core in range(N_CORES):
        o = res.results[core]["out"]  # [per, s, 128]
        for i, (bi, hi) in enumerate(units[core * per : (core + 1) * per]):
            full[bi, :, hi, :] = o[i]
    return full
